# revision 1
# baseline (speedup 1.0000x reference)
"""GATv2 (3-layer) Trainium2 Bass kernel, 8-core SPMD.

Strategy
--------
- Nodes sharded 2500/core (core k owns nodes [2500k, 2500k+2500)).
- Edges (incl. self-loops) sorted by dst and sharded by dst range, so
  segment-softmax + aggregation are core-local.
- Per layer:
    GEMM phase (data parallel over own nodes):  XL = H @ Wl, XR = H @ Wr
      (H kept feature-major "H^T" [F_in, 2500] so it directly feeds lhsT).
    AllGather XL shards -> XL_full [20000, HC] node-major (for src gathers).
    Edge phase: dst nodes processed in blocks of 128. For each block,
      T edge tiles of 128 edges:
        pass A: indirect-gather xl[src]; one-hot(dst) built via is_equal +
          PE transpose; s = xl[src] + xr[dst] via two matmuls into PSUM;
          LeakyReLU (ACT); logits = per-head <lr, att> (DVE ttr); w=exp(l-S);
          D[dst,h] += w (fp32 matmul accumulate over tiles).
        pass B: alpha = w * (1/D)[dst] (PE bcast of 1/D back to edges);
          U[dst,:] += onehot^T @ (alpha_h * xl[src]) (fp16 matmul, fp32 PSUM).
      Epilogue: U -> transpose chunks -> Relu(U + bias) -> H^T (f16, DRAM).
- Final: out = H3 @ Wf + bf, node-sharded; host concatenates 8 shards.

Numerics: features/weights fp16, PSUM fp32, logits/exp/denominator fp32,
alpha in [0,1] fp32 -> fp16 scaling. Validated ~8e-4 absmax-rel vs fp32 ref.
"""
import sys
if '/opt/trn_rl_repo' not in sys.path:
    sys.path.insert(0, '/opt/trn_rl_repo')

from dataclasses import dataclass
import numpy as np

import concourse.bass as bass
import concourse.bacc as bacc
import concourse.tile as tile
from concourse import mybir
from concourse import bass_utils
from concourse.masks import make_identity

P = 128
F32 = mybir.dt.float32
F16 = mybir.dt.float16
I32 = mybir.dt.int32

EXP_SHIFT = 4.0  # logits measured in [-6.1, 5.4]; any constant is exact math-wise


@dataclass
class LayerCfg:
    f_in: int
    heads: int
    out_ch: int

    @property
    def hc(self):
        return self.heads * self.out_ch

    @property
    def hca(self):
        return self.heads * self.out_ch + self.heads


@dataclass
class GatCfg:
    n_cores: int = 8
    shard: int = 2500          # nodes per core
    T: int = 18                # edge tiles per dst block
    layers: tuple = (LayerCfg(64, 3, 64), LayerCfg(192, 3, 256), LayerCfg(768, 1, 512))
    f_final: int = 512
    edge_mode: str = 'full'   # full | dmagather | nogather | noedge
    ablate: tuple = ()
    kloop: int = 1             # static repetitions of the whole pipeline (timing)
    ag_mode: str = 'collective'  # collective | copy (copy: for single-core timeline sim)
    bufs_small: int = 2
    bufs_mm512: int = 2
    bufs_sb: int = 2
    bufs_blk: int = 2         # subset of: nottr notp notp2 nofp32mm noexp noscaleap noabs nobiasap

    @property
    def n_nodes(self):
        return self.n_cores * self.shard

    @property
    def nblk(self):
        return (self.shard + P - 1) // P

    @property
    def hc_max(self):
        return max(L.hc for L in self.layers)


def _chunks(total, step):
    out = []
    off = 0
    while off < total:
        sz = min(step, total - off)
        out.append((off, sz))
        off += sz
    return out


def build_gat(cfg: GatCfg):
    nc = bacc.Bacc("TRN2", target_bir_lowering=False, debug=False,
                   num_devices=cfg.n_cores)
    NB, T, SH = cfg.nblk, cfg.T, cfg.shard
    HCM = cfg.hc_max

    # ---------------- external tensors (per-core) ----------------
    srcs = nc.dram_tensor("srcs", [NB, P, T], I32, kind="ExternalInput").ap()
    dsts = nc.dram_tensor("dsts", [NB, P, T], I32, kind="ExternalInput").ap()
    xT = nc.dram_tensor("xT", [cfg.layers[0].f_in, SH], F16, kind="ExternalInput").ap()

    wl_d, wr_d, att_d, biasT_d = [], [], [], []
    for li, L in enumerate(cfg.layers):
        nkc = len(_chunks(L.hc, P))
        wl_d.append(nc.dram_tensor(f"wl{li}", [L.f_in, L.hca], F16, kind="ExternalInput").ap())
        wr_d.append(nc.dram_tensor(f"wr{li}", [L.f_in, L.hca], F16, kind="ExternalInput").ap())
        att_d.append(nc.dram_tensor(f"att{li}", [P, L.hc], F32, kind="ExternalInput").ap())
        biasT_d.append(nc.dram_tensor(f"biasT{li}", [P, nkc], F32, kind="ExternalInput").ap())
    nkf = len(_chunks(cfg.f_final, P))
    wf = nc.dram_tensor("wf", [P, nkf], F16, kind="ExternalInput").ap()
    bf_col = nc.dram_tensor("bf_col", [P, 1], F32, kind="ExternalInput").ap()

    out = nc.dram_tensor("out", [SH, 1], F32, kind="ExternalOutput").ap()

    with tile.TileContext(nc) as tc:
        with tc.tile_pool(name="const", bufs=1) as constp, \
             tc.tile_pool(name="wpool", bufs=1) as wpool, \
             tc.tile_pool(name="sb", bufs=cfg.bufs_sb) as sb, \
             tc.tile_pool(name="blk", bufs=cfg.bufs_blk) as blk, \
             tc.tile_pool(name="ps", bufs=1, space="PSUM") as ps, \
             tc.tile_pool(name="ps2", bufs=2, space="PSUM") as ps2, \
             tc.tile_pool(name="dram", bufs=1, space="DRAM") as dram:

            # ---------------- constants ----------------
            ident32 = constp.tile([P, P], F32, name="ident32")
            make_identity(nc, ident32[:])
            ident16 = constp.tile([P, P], F16, name="ident16")
            nc.vector.tensor_copy(out=ident16[:], in_=ident32[:])
            iota_i = constp.tile([P, P], I32, name="iota_i")
            nc.gpsimd.iota(iota_i[:], pattern=[[1, P]], base=0, channel_multiplier=0)
            iota_f = constp.tile([P, P], F32, name="iota_f")
            nc.vector.tensor_copy(out=iota_f[:], in_=iota_i[:])
            shift_col = constp.tile([P, 1], F32, name="shift_col")
            nc.gpsimd.memset(shift_col[:], -EXP_SHIFT)

            # resident weights / att / biasT
            wl_sb, wr_sb, att_sb, biasT_sb = [], [], [], []
            for li, L in enumerate(cfg.layers):
                wlk, wrk = [], []
                for ki, (ko, ks) in enumerate(_chunks(L.f_in, P)):
                    t1 = wpool.tile([ks, L.hca], F16, name=f"wl{li}k{ki}")
                    nc.sync.dma_start(out=t1[:], in_=wl_d[li][ko:ko + ks, :])
                    wlk.append(t1)
                    t2 = wpool.tile([ks, L.hca], F16, name=f"wr{li}k{ki}")
                    nc.sync.dma_start(out=t2[:], in_=wr_d[li][ko:ko + ks, :])
                    wrk.append(t2)
                wl_sb.append(wlk)
                wr_sb.append(wrk)
                ta = wpool.tile([P, L.hc], F32, name=f"att{li}")
                nc.sync.dma_start(out=ta[:], in_=att_d[li][:])
                att_sb.append(ta)
                nkc = len(_chunks(L.hc, P))
                tb = wpool.tile([P, nkc], F32, name=f"biasT{li}")
                nc.sync.dma_start(out=tb[:], in_=biasT_d[li][:])
                biasT_sb.append(tb)
            wf_sb = wpool.tile([P, nkf], F16, name="wf_sb")
            nc.sync.dma_start(out=wf_sb[:], in_=wf[:])
            bf_sb = wpool.tile([P, 1], F32, name="bf_sb")
            nc.sync.dma_start(out=bf_sb[:], in_=bf_col[:])

            # per-layer DRAM scratch (rebuilt per kloop rep)
            hT_dram = []

            # =========================================================
            def gemm_phase(li):
                L = cfg.layers[li]
                kcs = _chunks(L.f_in, P)
                ncs = _chunks(L.hca, 512)
                ag_in = dram.tile([SH, L.hca], F16, name=f"ag_in{li}")
                xr_sh = dram.tile([NB * P, L.hca], F16, name=f"xr{li}")
                pad = NB * P - SH
                if pad:
                    ztile = sb.tile([pad, L.hca], F16, name="zpad", tag="zpad", bufs=1)
                    nc.gpsimd.memset(ztile[:], 0.0)
                    nc.sync.dma_start(out=xr_sh[SH:NB * P, :], in_=ztile[:])
                src_ap = xT if li == 0 else hT_dram[li - 1]
                for m in range(NB):
                    mo = m * P
                    mn = min(P, SH - mo)
                    lhs = []
                    for ki, (ko, ks) in enumerate(kcs):
                        lt = sb.tile([ks, P], F16, name="lhsT", tag=f"lhsT{ki}")
                        nc.sync.dma_start(out=lt[:, :mn], in_=src_ap[ko:ko + ks, mo:mo + mn])
                        lhs.append(lt)
                    for wsb, dst_d in ((wl_sb[li], ag_in), (wr_sb[li], xr_sh)):
                        og = sb.tile([P, L.hca], F16, name="og", tag="og")
                        for (no, ns) in ncs:
                            pg = ps2.tile([P, ns], F32, name="pg", tag="mm512", bufs=cfg.bufs_mm512)
                            for ki in range(len(kcs)):
                                nc.tensor.matmul(
                                    out=pg[:mn, :ns],
                                    lhsT=lhs[ki][:, :mn],
                                    rhs=wsb[ki][:, no:no + ns],
                                    start=(ki == 0), stop=(ki == len(kcs) - 1))
                            nc.scalar.copy(out=og[:mn, no:no + ns], in_=pg[:mn, :ns])
                        nc.sync.dma_start(out=dst_d[mo:mo + mn, :], in_=og[:mn, :])
                return ag_in, xr_sh

            # =========================================================
            def edge_phase(li, xl_full, xr_sh):
                L = cfg.layers[li]
                H, C, HC, HCA = L.heads, L.out_ch, L.hc, L.hca
                ncs = _chunks(HCA, 512)
                for b in range(NB):
                    bn = min(P, SH - b * P)
                    src_i = blk.tile([P, T], I32, name="src_i", tag="src_i")
                    nc.sync.dma_start(out=src_i[:], in_=srcs[b, :, :])
                    dst_i = blk.tile([P, T], I32, name="dst_i", tag="dst_i")
                    nc.sync.dma_start(out=dst_i[:], in_=dsts[b, :, :])
                    dst_f = blk.tile([P, T], F32, name="dst_f", tag="dst_f")
                    nc.vector.tensor_copy(out=dst_f[:], in_=dst_i[:])
                    xr_blk = blk.tile([P, HCA], F16, name="xr_blk", tag="xr_blk")
                    nc.sync.dma_start(out=xr_blk[:], in_=xr_sh[b * P:(b + 1) * P, :])

                    xl_all = blk.tile([P, T * HCA], F16, name="xl_all", tag="xl_all")
                    oh16_all = blk.tile([P, T * P], F16, name="oh16_all", tag="oh16_all")
                    oh32_all = blk.tile([P, T * P], F32, name="oh32_all", tag="oh32_all")
                    de16_all = blk.tile([P, T * P], F16, name="de16_all", tag="de16_all")
                    de32_all = blk.tile([P, T * P], F32, name="de32_all", tag="de32_all")
                    w_all = blk.tile([P, T * H], F32, name="w_all", tag="w_all")
                    logit_all = blk.tile([P, T * H], F32, name="logit_all", tag="logit_all")

                    d_ps = ps.tile([P, H], F32, name="d_ps", tag="d_ps")
                    u_ps = ps.tile([P, HC], F32, name="u_ps", tag="u_ps")

                    # ---------------- pass A ----------------
                    for t in range(T):
                        xl_g = xl_all[:, t * HCA:(t + 1) * HCA]
                        if cfg.edge_mode == 'full':
                            nc.gpsimd.indirect_dma_start(
                                out=xl_g, out_offset=None, in_=xl_full[:],
                                in_offset=bass.IndirectOffsetOnAxis(ap=src_i[:, t:t + 1], axis=0))
                        elif cfg.edge_mode == 'dmagather':
                            nc.gpsimd.dma_start(out=xl_g, in_=xl_full[(b % 100) * P:(b % 100) * P + P, :])
                        else:
                            nc.gpsimd.memset(xl_g, 0.01)
                        oh16 = oh16_all[:, t * P:(t + 1) * P]
                        oh32 = oh32_all[:, t * P:(t + 1) * P]
                        nc.vector.tensor_scalar(
                            out=oh32, in0=iota_f[:], scalar1=dst_f[:, t:t + 1],
                            scalar2=None, op0=mybir.AluOpType.is_equal)
                        nc.vector.tensor_scalar(
                            out=oh16, in0=iota_f[:], scalar1=dst_f[:, t:t + 1],
                            scalar2=None, op0=mybir.AluOpType.is_equal)
                        de16 = de16_all[:, t * P:(t + 1) * P]
                        de32 = de32_all[:, t * P:(t + 1) * P]
                        if 'notp' in cfg.ablate:
                            nc.scalar.copy(out=de16, in_=ident32[:])
                            nc.vector.tensor_copy(out=de32, in_=ident32[:])
                        else:
                            tp = ps2.tile([P, P], F32, name="tp", tag="small_ps", bufs=cfg.bufs_small)
                            nc.tensor.transpose(out=tp[:], in_=oh32, identity=ident32[:])
                            nc.scalar.copy(out=de16, in_=tp[:])
                            nc.vector.tensor_copy(out=de32, in_=tp[:])
                        lr = sb.tile([P, HC], F32, name="lr", tag="lr")
                        lin06 = sb.tile([P, H], F32, name="lin06", tag="lin06")
                        for (no, ns) in ncs:
                            sp = ps2.tile([P, ns], F32, name="sp", tag="mm512", bufs=cfg.bufs_mm512)
                            if 'nos' in cfg.ablate:
                                nc.tensor.matmul(out=sp[:], lhsT=ident16[:], rhs=xl_g[:, no:no + ns],
                                                 start=True, stop=True)
                            else:
                                nc.tensor.matmul(out=sp[:], lhsT=de16, rhs=xr_blk[:, no:no + ns],
                                                 start=True, stop=False)
                                nc.tensor.matmul(out=sp[:], lhsT=ident16[:], rhs=xl_g[:, no:no + ns],
                                                 start=False, stop=True)
                            if no < HC:
                                an = min(ns, HC - no)
                                nc.scalar.activation(out=lr[:, no:no + an], in_=sp[:, :an],
                                                     func=(mybir.ActivationFunctionType.Copy
                                                           if 'noabs' in cfg.ablate else
                                                           mybir.ActivationFunctionType.Abs))
                            if no + ns > HC:
                                # linear logit columns HC..HCA, scaled by 0.6
                                lo = max(HC - no, 0)
                                nc.scalar.activation(out=lin06[:, :H], in_=sp[:, lo:lo + H],
                                                     func=mybir.ActivationFunctionType.Copy,
                                                     scale=0.6)
                        # logits = sum_c (0.4*att)|s| + 0.6*lin  (att pre-scaled on host)
                        if 'nologits' in cfg.ablate:
                            nc.vector.tensor_copy(out=logit_all[:, t * H:(t + 1) * H],
                                                  in_=lin06[:, :H])
                        else:
                            prod = sb.tile([P, HC], F32, name="prod", tag="prod")
                            nc.vector.tensor_tensor(out=prod[:], in0=lr[:], in1=att_sb[li][:, :HC],
                                                    op=mybir.AluOpType.mult)
                            red = sb.tile([P, H], F32, name="red", tag="red")
                            nc.vector.tensor_reduce(
                                out=red[:], in_=prod[:].rearrange("p (h c) -> p h c", h=H),
                                axis=mybir.AxisListType.X, op=mybir.AluOpType.add)
                            nc.vector.tensor_tensor(out=logit_all[:, t * H:(t + 1) * H],
                                                    in0=red[:], in1=lin06[:, :H],
                                                    op=mybir.AluOpType.add)
                        if 'noexp' in cfg.ablate:
                            nc.vector.tensor_copy(out=w_all[:, t * H:(t + 1) * H],
                                                  in_=logit_all[:, t * H:(t + 1) * H])
                        else:
                            nc.scalar.activation(out=w_all[:, t * H:(t + 1) * H],
                                                 in_=logit_all[:, t * H:(t + 1) * H],
                                                 func=mybir.ActivationFunctionType.Exp,
                                                 bias=shift_col[:])
                        if 'nofp32mm' in cfg.ablate:
                            w16 = sb.tile([P, H], F16, name="w16", tag="w16")
                            nc.vector.tensor_copy(out=w16[:], in_=w_all[:, t * H:(t + 1) * H])
                            nc.tensor.matmul(out=d_ps[:, :H], lhsT=oh16,
                                             rhs=w16[:],
                                             start=(t == 0), stop=(t == T - 1))
                        else:
                            nc.tensor.matmul(out=d_ps[:, :H], lhsT=oh32,
                                             rhs=w_all[:, t * H:(t + 1) * H],
                                             start=(t == 0), stop=(t == T - 1))
                    dsb = sb.tile([P, H], F32, name="dsb", tag="dsb")
                    nc.vector.tensor_scalar(out=dsb[:], in0=d_ps[:, :H], scalar1=1e-30,
                                            scalar2=None, op0=mybir.AluOpType.add)
                    recip = sb.tile([P, H], F32, name="recip", tag="recip")
                    nc.vector.reciprocal(out=recip[:], in_=dsb[:])

                    # ---------------- pass B ----------------
                    for t in range(T):
                        xl_g = xl_all[:, t * HCA:(t + 1) * HCA]
                        ap_ps = ps2.tile([P, H], F32, name="ap_ps", tag="small_ps", bufs=cfg.bufs_small)
                        if 'nofp32mm' in cfg.ablate:
                            recip16 = sb.tile([P, H], F16, name="recip16", tag="recip16")
                            nc.vector.tensor_copy(out=recip16[:], in_=recip[:])
                            nc.tensor.matmul(out=ap_ps[:, :H], lhsT=de16_all[:, t * P:(t + 1) * P],
                                             rhs=recip16[:], start=True, stop=True)
                        else:
                            nc.tensor.matmul(out=ap_ps[:, :H], lhsT=de32_all[:, t * P:(t + 1) * P],
                                             rhs=recip[:], start=True, stop=True)
                        alpha = sb.tile([P, H], F32, name="alpha", tag="alpha")
                        nc.vector.tensor_tensor(out=alpha[:], in0=w_all[:, t * H:(t + 1) * H],
                                                in1=ap_ps[:, :H], op=mybir.AluOpType.mult)
                        rsc = sb.tile([P, HC], F16, name="rsc", tag="rsc")
                        for h in range(H):
                            nc.scalar.activation(
                                out=rsc[:, h * C:(h + 1) * C], in_=xl_g[:, h * C:(h + 1) * C],
                                func=mybir.ActivationFunctionType.Copy,
                                scale=(0.5 if 'noscaleap' in cfg.ablate else alpha[:, h:h + 1]))
                        for (no, ns) in _chunks(HC, 512):
                            if 'nou' in cfg.ablate and t > 0:
                                continue
                            nc.tensor.matmul(out=u_ps[:, no:no + ns],
                                             lhsT=oh16_all[:, t * P:(t + 1) * P],
                                             rhs=rsc[:, no:no + ns],
                                             start=(t == 0), stop=('nou' in cfg.ablate or t == T - 1))

                    # ---------------- epilogue ----------------
                    u_sb = sb.tile([P, HC], F32, name="u_sb", tag="u_sb")
                    for (no, ns) in _chunks(HC, 512):
                        nc.vector.tensor_copy(out=u_sb[:, no:no + ns], in_=u_ps[:, no:no + ns])
                    for kc, (fo, fs) in enumerate(_chunks(HC, P)):
                        tp2 = ps2.tile([P, P], F32, name="tp2", tag="small_ps", bufs=cfg.bufs_small)
                        if 'notp2' in cfg.ablate:
                            nc.vector.tensor_copy(out=tp2[:fs, :fs], in_=u_sb[:fs, fo:fo + fs])
                        else:
                            nc.tensor.transpose(out=tp2[:fs, :], in_=u_sb[:, fo:fo + fs],
                                                identity=ident32[:])
                        hts = sb.tile([P, P], F16, name="hts", tag="hts")
                        nc.scalar.activation(out=hts[:fs, :bn], in_=tp2[:fs, :bn],
                                             func=mybir.ActivationFunctionType.Relu,
                                             bias=(0.0 if 'nobiasap' in cfg.ablate
                                                   else biasT_sb[li][:fs, kc:kc + 1]),
                                             scale=1.0)
                        nc.sync.dma_start(
                            out=hT_dram[li][fo:fo + fs, b * P:b * P + bn],
                            in_=hts[:fs, :bn])

            # =========================================================
            for rep in range(cfg.kloop):
                hT_dram.clear()
                for li, L in enumerate(cfg.layers):
                    hT_dram.append(dram.tile([L.hc, SH], F16, name=f"hT{li}r{rep}"))
                for li, L in enumerate(cfg.layers):
                    ag_in, xr_sh = gemm_phase(li)
                    if cfg.ag_mode == 'collective':
                        xl_full = dram.tile([cfg.n_nodes, L.hca], F16, name=f"xl_full{li}",
                                            addr_space="Shared")
                        nc.gpsimd.collective_compute(
                            "AllGather", mybir.AluOpType.bypass,
                            replica_groups=[list(range(cfg.n_cores))],
                            ins=[ag_in[:]], outs=[xl_full[:]])
                    else:
                        xl_full = dram.tile([cfg.n_nodes, L.hca], F16, name=f"xl_full{li}")
                        for r in range(cfg.n_nodes // SH):
                            nc.sync.dma_start(out=xl_full[r * SH:(r + 1) * SH, :], in_=ag_in[:])
                    if cfg.edge_mode != 'noedge':
                        edge_phase(li, xl_full, xr_sh)

            # final linear
            kcs = _chunks(cfg.f_final, P)
            for m in range(NB):
                mo = m * P
                mn = min(P, SH - mo)
                pf = ps2.tile([P, 1], F32, name="pf", tag="small_ps", bufs=cfg.bufs_small)
                lhs = []
                for ki, (ko, ks) in enumerate(kcs):
                    lt = sb.tile([ks, P], F16, name="lhsTf", tag=f"lhsTf{ki}")
                    nc.sync.dma_start(out=lt[:, :mn], in_=hT_dram[-1][ko:ko + ks, mo:mo + mn])
                    lhs.append(lt)
                for ki, (ko, ks) in enumerate(kcs):
                    nc.tensor.matmul(out=pf[:mn, :], lhsT=lhs[ki][:, :mn],
                                     rhs=wf_sb[:ks, ki:ki + 1],
                                     start=(ki == 0), stop=(ki == len(kcs) - 1))
                of = sb.tile([P, 1], F32, name="of", tag="of")
                nc.scalar.activation(out=of[:mn, :], in_=pf[:mn, :],
                                     func=mybir.ActivationFunctionType.Identity,
                                     bias=bf_sb[:mn, :], scale=1.0)
                nc.sync.dma_start(out=out[mo:mo + mn, :], in_=of[:mn, :])

    nc.compile()
    return nc


# =====================================================================
# host-side data prep
# =====================================================================

def prep_host(inputs, cfg: GatCfg):
    N, SH, NB = cfg.n_nodes, cfg.shard, cfg.nblk
    x = np.asarray(inputs['x'], dtype=np.float32)
    ei = np.asarray(inputs['edge_index']).astype(np.int64)
    loop = np.arange(N, dtype=np.int64)
    src = np.concatenate([ei[0], loop])
    dst = np.concatenate([ei[1], loop])
    order = np.argsort(dst, kind='stable')
    src_s, dst_s = src[order], dst[order]

    cnt = np.zeros((cfg.n_cores, NB), dtype=np.int64)
    bounds = {}
    for c in range(cfg.n_cores):
        for b in range(NB):
            blk_lo = c * SH + b * P
            blk_hi = min(blk_lo + P, (c + 1) * SH)
            lo = np.searchsorted(dst_s, blk_lo)
            hi = np.searchsorted(dst_s, blk_hi)
            bounds[(c, b)] = (lo, hi, blk_lo)
            cnt[c, b] = hi - lo
    T = int((cnt.max() + P - 1) // P)
    cfg.T = T

    in_maps = []
    for c in range(cfg.n_cores):
        srcs = np.zeros((NB, P, T), dtype=np.int32)
        dsts = np.full((NB, P, T), -1, dtype=np.int32)
        for b in range(NB):
            lo, hi, blk_lo = bounds[(c, b)]
            ne = hi - lo
            s = np.zeros(T * P, dtype=np.int32)
            d = np.full(T * P, -1, dtype=np.int32)
            s[:ne] = src_s[lo:hi]
            d[:ne] = (dst_s[lo:hi] - blk_lo)
            srcs[b] = s.reshape(T, P).T
            dsts[b] = d.reshape(T, P).T
        xT = np.ascontiguousarray(x[c * SH:(c + 1) * SH, :].T).astype(np.float16)
        im = {'srcs': srcs, 'dsts': dsts, 'xT': xT}
        for li, L in enumerate(cfg.layers):
            Wl = np.asarray(inputs[f'Wl{li + 1}'], np.float32)
            Wr = np.asarray(inputs[f'Wr{li + 1}'], np.float32)
            att = np.asarray(inputs[f'att{li + 1}'], np.float32).reshape(1, -1)
            bias = np.asarray(inputs[f'b{li + 1}'], np.float32).reshape(-1)
            nkc = len(_chunks(L.hc, P))
            bT = np.zeros((P, nkc), dtype=np.float32)
            for kc, (fo, fs) in enumerate(_chunks(L.hc, P)):
                bT[:fs, kc] = bias[fo:fo + fs]
            att_bd = np.zeros((L.hc, L.heads), dtype=np.float32)
            for h in range(L.heads):
                att_bd[h * L.out_ch:(h + 1) * L.out_ch, h] = \
                    np.asarray(inputs[f'att{li + 1}'], np.float32)[h]
            wl_aug = np.concatenate([Wl, Wl @ att_bd], axis=1)
            wr_aug = np.concatenate([Wr, Wr @ att_bd], axis=1)
            im[f'wl{li}'] = wl_aug.astype(np.float16)
            im[f'wr{li}'] = wr_aug.astype(np.float16)
            im[f'att{li}'] = np.repeat(att * 0.4, P, axis=0).astype(np.float32)
            im[f'biasT{li}'] = bT
        wf_flat = np.asarray(inputs['Wf'], np.float32).reshape(-1)
        nkf = len(_chunks(cfg.f_final, P))
        wfp = np.zeros((P, nkf), dtype=np.float32)
        for ki, (ko, ks) in enumerate(_chunks(cfg.f_final, P)):
            wfp[:ks, ki] = wf_flat[ko:ko + ks]
        im['wf'] = wfp.astype(np.float16)
        im['bf_col'] = np.full((P, 1), np.asarray(inputs['bf'], np.float32).reshape(-1)[0],
                               dtype=np.float32)
        in_maps.append(im)
    return in_maps, T


_CACHE = {}


def kernel(**inputs) -> np.ndarray:
    cfg = GatCfg()
    in_maps, T = prep_host(inputs, cfg)
    key = ('full', T)
    if key not in _CACHE:
        _CACHE[key] = build_gat(cfg)
    nc = _CACHE[key]
    res = bass_utils.run_bass_kernel_spmd(nc, in_maps, core_ids=list(range(cfg.n_cores)))
    out = np.concatenate([res.results[c]['out'] for c in range(cfg.n_cores)], axis=0)
    return out.astype(np.float32)



# revision 3
# speedup vs baseline: 1.3613x; 1.3613x over previous
"""GATv2 (3-layer) Trainium2 Bass kernel, 8-core SPMD — v2.

Strategy
--------
- Nodes sharded 2500/core; edges (incl. self-loops) sorted by dst and
  sharded by dst range, so segment-softmax + aggregation are core-local.
- Per layer:
    GEMM phase (data parallel over own nodes): XL = H @ Wl_aug,
      XR = H @ Wr_aug (aug cols carry 0.6*att.s linear-logit terms).
    AllGather XL shards -> XL_full [20000, HCA] (for src gathers).
    Edge phase: dst blocks of 128 nodes x T edge tiles of 128 edges.
      One-hot matrices oh16 (edge-major) / de16 (dst-major) are STATIC
      per graph — built on host, streamed from DRAM per block.
      Per tile:
        indirect-gather xl[src]; sp = de16 @ xr + I @ xl (PSUM);
        lr = |sp| (ACT Abs); prod = lr * att (DVE f16);
        red = per-head sum (DVE reduce); logit = red + sp_aug (DVE);
        w = exp(logit - SHIFT) (ACT); rsc = [w (.) xl | w16];
        u_ps[:, 0:HC+H] += oh16^T @ rsc  (accumulates both the
          unnormalized weighted sum AND the softmax denominator D).
      Epilogue per block: recip(D); u16 = u_ps * recip_h (ACT, PSUM
      read); transpose chunks (PE, f16); Relu(u + bias) -> hT (DRAM).
- Final: out = H3 @ Wf + bf, node-sharded; host concatenates 8 shards.

Numerics: features/weights fp16, PSUM fp32, logits fp32, exp shifted
by -4 so w = exp(logit-4) <= ~4 stays in fp16 range. alpha = w/D is
applied after aggregation (exactly equal to normalizing per edge).
"""
import sys
if '/opt/trn_rl_repo' not in sys.path:
    sys.path.insert(0, '/opt/trn_rl_repo')

from dataclasses import dataclass
import numpy as np

import concourse.bass as bass
import concourse.bacc as bacc
import concourse.tile as tile
from concourse import mybir
from concourse import bass_utils
from concourse.masks import make_identity

P = 128
F32 = mybir.dt.float32
F16 = mybir.dt.float16
I32 = mybir.dt.int32

EXP_SHIFT = 4.0  # logits measured in [-6.1, 5.4]; any constant is exact math-wise


@dataclass
class LayerCfg:
    f_in: int
    heads: int
    out_ch: int

    @property
    def hc(self):
        return self.heads * self.out_ch

    @property
    def hca(self):
        return self.heads * self.out_ch + self.heads


@dataclass
class GatCfg:
    n_cores: int = 8
    shard: int = 2500          # nodes per core
    T: int = 18                # edge tiles per dst block
    layers: tuple = (LayerCfg(64, 3, 64), LayerCfg(192, 3, 256), LayerCfg(768, 1, 512))
    f_final: int = 512
    bufs_sp: int = 2           # PSUM bufs for sp tiles
    bufs_xl: int = 4           # gather destination bufs
    bufs_sb: int = 2
    bufs_blk: int = 2

    @property
    def n_nodes(self):
        return self.n_cores * self.shard

    @property
    def nblk(self):
        return (self.shard + P - 1) // P

    @property
    def hc_max(self):
        return max(L.hc for L in self.layers)


def _chunks(total, step):
    out = []
    off = 0
    while off < total:
        sz = min(step, total - off)
        out.append((off, sz))
        off += sz
    return out


def build_gat(cfg: GatCfg):
    nc = bacc.Bacc("TRN2", target_bir_lowering=False, debug=False,
                   num_devices=cfg.n_cores)
    NB, T, SH = cfg.nblk, cfg.T, cfg.shard

    # ---------------- external tensors (per-core) ----------------
    srcs = nc.dram_tensor("srcs", [NB, P, T], I32, kind="ExternalInput").ap()
    oh16_d = nc.dram_tensor("oh16", [NB, P, T * P], F16, kind="ExternalInput").ap()
    de16_d = nc.dram_tensor("de16", [NB, P, T * P], F16, kind="ExternalInput").ap()
    xT = nc.dram_tensor("xT", [cfg.layers[0].f_in, SH], F16, kind="ExternalInput").ap()

    wl_d, wr_d, att_d, biasT_d = [], [], [], []
    for li, L in enumerate(cfg.layers):
        nkc = len(_chunks(L.hc, P))
        wl_d.append(nc.dram_tensor(f"wl{li}", [L.f_in, L.hca], F16, kind="ExternalInput").ap())
        wr_d.append(nc.dram_tensor(f"wr{li}", [L.f_in, L.hca], F16, kind="ExternalInput").ap())
        att_d.append(nc.dram_tensor(f"att{li}", [P, L.hc], F16, kind="ExternalInput").ap())
        biasT_d.append(nc.dram_tensor(f"biasT{li}", [P, nkc], F32, kind="ExternalInput").ap())
    nkf = len(_chunks(cfg.f_final, P))
    wf = nc.dram_tensor("wf", [P, nkf], F16, kind="ExternalInput").ap()
    bf_col = nc.dram_tensor("bf_col", [P, 1], F32, kind="ExternalInput").ap()

    out = nc.dram_tensor("out", [SH, 1], F32, kind="ExternalOutput").ap()

    with tile.TileContext(nc) as tc:
        with tc.tile_pool(name="const", bufs=1) as constp, \
             tc.tile_pool(name="wpool", bufs=1) as wpool, \
             tc.tile_pool(name="sb", bufs=cfg.bufs_sb) as sb, \
             tc.tile_pool(name="blk", bufs=cfg.bufs_blk) as blk, \
             tc.tile_pool(name="ps", bufs=1, space="PSUM") as ps, \
             tc.tile_pool(name="ps2", bufs=2, space="PSUM") as ps2, \
             tc.tile_pool(name="dram", bufs=1, space="DRAM") as dram:

            # ---------------- constants ----------------
            ident32 = constp.tile([P, P], F32, name="ident32")
            make_identity(nc, ident32[:])
            ident16 = constp.tile([P, P], F16, name="ident16")
            nc.vector.tensor_copy(out=ident16[:], in_=ident32[:])
            shift_col = constp.tile([P, 1], F32, name="shift_col")
            nc.vector.memset(shift_col[:], -EXP_SHIFT)

            # resident weights / att / biasT
            wl_sb, wr_sb, att_sb, biasT_sb = [], [], [], []
            for li, L in enumerate(cfg.layers):
                wlk, wrk = [], []
                for ki, (ko, ks) in enumerate(_chunks(L.f_in, P)):
                    t1 = wpool.tile([ks, L.hca], F16, name=f"wl{li}k{ki}")
                    nc.sync.dma_start(out=t1[:], in_=wl_d[li][ko:ko + ks, :])
                    wlk.append(t1)
                    t2 = wpool.tile([ks, L.hca], F16, name=f"wr{li}k{ki}")
                    nc.sync.dma_start(out=t2[:], in_=wr_d[li][ko:ko + ks, :])
                    wrk.append(t2)
                wl_sb.append(wlk)
                wr_sb.append(wrk)
                ta = wpool.tile([P, L.hc], F16, name=f"att{li}")
                nc.sync.dma_start(out=ta[:], in_=att_d[li][:])
                att_sb.append(ta)
                nkc = len(_chunks(L.hc, P))
                tb = wpool.tile([P, nkc], F32, name=f"biasT{li}")
                nc.sync.dma_start(out=tb[:], in_=biasT_d[li][:])
                biasT_sb.append(tb)
            wf_sb = wpool.tile([P, nkf], F16, name="wf_sb")
            nc.sync.dma_start(out=wf_sb[:], in_=wf[:])
            bf_sb = wpool.tile([P, 1], F32, name="bf_sb")
            nc.sync.dma_start(out=bf_sb[:], in_=bf_col[:])

            hT_dram = []

            # =========================================================
            def gemm_phase(li):
                L = cfg.layers[li]
                kcs = _chunks(L.f_in, P)
                ncs = _chunks(L.hca, 512)
                ag_in = dram.tile([SH, L.hca], F16, name=f"ag_in{li}")
                xr_sh = dram.tile([NB * P, L.hca], F16, name=f"xr{li}")
                pad = NB * P - SH
                if pad:
                    ztile = sb.tile([pad, L.hca], F16, name="zpad", tag="zpad", bufs=1)
                    nc.vector.memset(ztile[:], 0.0)
                    nc.sync.dma_start(out=xr_sh[SH:NB * P, :], in_=ztile[:])
                src_ap = xT if li == 0 else hT_dram[li - 1]
                for m in range(NB):
                    mo = m * P
                    mn = min(P, SH - mo)
                    lhs = []
                    for ki, (ko, ks) in enumerate(kcs):
                        lt = sb.tile([ks, P], F16, name="lhsT", tag=f"lhsT{ki}")
                        nc.sync.dma_start(out=lt[:, :mn], in_=src_ap[ko:ko + ks, mo:mo + mn])
                        lhs.append(lt)
                    for wsb, dst_d in ((wl_sb[li], ag_in), (wr_sb[li], xr_sh)):
                        og = sb.tile([P, L.hca], F16, name="og", tag="og")
                        for (no, ns) in ncs:
                            pg = ps2.tile([P, ns], F32, name="pg", tag="mm512", bufs=2)
                            for ki in range(len(kcs)):
                                nc.tensor.matmul(
                                    out=pg[:mn, :ns],
                                    lhsT=lhs[ki][:, :mn],
                                    rhs=wsb[ki][:, no:no + ns],
                                    start=(ki == 0), stop=(ki == len(kcs) - 1))
                            nc.scalar.copy(out=og[:mn, no:no + ns], in_=pg[:mn, :ns])
                        nc.sync.dma_start(out=dst_d[mo:mo + mn, :], in_=og[:mn, :])
                return ag_in, xr_sh

            # =========================================================
            def edge_phase(li, xl_full, xr_sh):
                L = cfg.layers[li]
                H, C, HC, HCA = L.heads, L.out_ch, L.hc, L.hca
                ncs = _chunks(HCA, 512)
                ucs = _chunks(HC + H, 512)
                for b in range(NB):
                    bn = min(P, SH - b * P)
                    src_i = blk.tile([P, T], I32, name="src_i", tag="src_i")
                    nc.sync.dma_start(out=src_i[:], in_=srcs[b, :, :])
                    oh_b = blk.tile([P, T * P], F16, name="oh_b", tag="oh_b")
                    nc.sync.dma_start(out=oh_b[:], in_=oh16_d[b, :, :])
                    de_b = blk.tile([P, T * P], F16, name="de_b", tag="de_b")
                    nc.sync.dma_start(out=de_b[:], in_=de16_d[b, :, :])
                    xr_blk = blk.tile([P, HCA], F16, name="xr_blk", tag="xr_blk")
                    nc.sync.dma_start(out=xr_blk[:], in_=xr_sh[b * P:(b + 1) * P, :])

                    u_ps = ps.tile([P, HC + H], F32, name="u_ps", tag="u_ps")

                    for t in range(T):
                        xl_g = sb.tile([P, HCA], F16, name="xl_g", tag="xl_g",
                                       bufs=cfg.bufs_xl)
                        nc.gpsimd.indirect_dma_start(
                            out=xl_g[:], out_offset=None, in_=xl_full[:],
                            in_offset=bass.IndirectOffsetOnAxis(ap=src_i[:, t:t + 1], axis=0))
                        sp = ps2.tile([P, HCA], F32, name="sp", tag="mm512",
                                      bufs=cfg.bufs_sp)
                        for (no, ns) in ncs:
                            nc.tensor.matmul(out=sp[:, no:no + ns],
                                             lhsT=de_b[:, t * P:(t + 1) * P],
                                             rhs=xr_blk[:, no:no + ns],
                                             start=True, stop=False)
                            nc.tensor.matmul(out=sp[:, no:no + ns],
                                             lhsT=ident16[:],
                                             rhs=xl_g[:, no:no + ns],
                                             start=False, stop=True)
                        lr = sb.tile([P, HC], F16, name="lr", tag="lr")
                        nc.scalar.activation(out=lr[:], in_=sp[:, :HC],
                                             func=mybir.ActivationFunctionType.Abs)
                        prod = sb.tile([P, HC], F16, name="prod", tag="prod")
                        nc.vector.tensor_tensor(out=prod[:], in0=lr[:],
                                                in1=att_sb[li][:],
                                                op=mybir.AluOpType.mult)
                        red = sb.tile([P, H], F32, name="red", tag="red")
                        nc.vector.tensor_reduce(
                            out=red[:], in_=prod[:].rearrange("p (h c) -> p h c", h=H),
                            axis=mybir.AxisListType.X, op=mybir.AluOpType.add)
                        logit = sb.tile([P, H], F32, name="logit", tag="logit")
                        nc.vector.tensor_tensor(out=logit[:], in0=red[:],
                                                in1=sp[:, HC:HC + H],
                                                op=mybir.AluOpType.add)
                        w32 = sb.tile([P, H], F32, name="w32", tag="w32")
                        nc.scalar.activation(out=w32[:], in_=logit[:],
                                             func=mybir.ActivationFunctionType.Exp,
                                             bias=shift_col[:])
                        rsc = sb.tile([P, HC + H], F16, name="rsc", tag="rsc", bufs=3)
                        for h in range(H):
                            nc.scalar.activation(
                                out=rsc[:, h * C:(h + 1) * C], in_=xl_g[:, h * C:(h + 1) * C],
                                func=mybir.ActivationFunctionType.Copy,
                                scale=w32[:, h:h + 1])
                        nc.vector.tensor_copy(out=rsc[:, HC:HC + H], in_=w32[:])
                        for (no, ns) in ucs:
                            nc.tensor.matmul(out=u_ps[:, no:no + ns],
                                             lhsT=oh_b[:, t * P:(t + 1) * P],
                                             rhs=rsc[:, no:no + ns],
                                             start=(t == 0), stop=(t == T - 1))

                    # ---------------- epilogue ----------------
                    dsb = sb.tile([P, H], F32, name="dsb", tag="dsb")
                    nc.vector.tensor_scalar(out=dsb[:], in0=u_ps[:, HC:HC + H],
                                            scalar1=1e-30, scalar2=None,
                                            op0=mybir.AluOpType.add)
                    recip = sb.tile([P, H], F32, name="recip", tag="recip")
                    nc.vector.reciprocal(out=recip[:], in_=dsb[:])
                    u16 = sb.tile([P, HC], F16, name="u16", tag="u16")
                    for h in range(H):
                        nc.scalar.activation(
                            out=u16[:, h * C:(h + 1) * C], in_=u_ps[:, h * C:(h + 1) * C],
                            func=mybir.ActivationFunctionType.Copy,
                            scale=recip[:, h:h + 1])
                    for kc, (fo, fs) in enumerate(_chunks(HC, P)):
                        tp2 = ps2.tile([P, P], F16, name="tp2", tag="small_ps", bufs=2)
                        nc.tensor.transpose(out=tp2[:fs, :], in_=u16[:, fo:fo + fs],
                                            identity=ident16[:])
                        hts = sb.tile([P, P], F16, name="hts", tag="hts")
                        nc.scalar.activation(out=hts[:fs, :bn], in_=tp2[:fs, :bn],
                                             func=mybir.ActivationFunctionType.Relu,
                                             bias=biasT_sb[li][:fs, kc:kc + 1],
                                             scale=1.0)
                        nc.sync.dma_start(
                            out=hT_dram[li][fo:fo + fs, b * P:b * P + bn],
                            in_=hts[:fs, :bn])

            # =========================================================
            for li, L in enumerate(cfg.layers):
                hT_dram.append(dram.tile([L.hc, SH], F16, name=f"hT{li}"))
            for li, L in enumerate(cfg.layers):
                ag_in, xr_sh = gemm_phase(li)
                xl_full = dram.tile([cfg.n_nodes, L.hca], F16, name=f"xl_full{li}",
                                    addr_space="Shared")
                nc.gpsimd.collective_compute(
                    "AllGather", mybir.AluOpType.bypass,
                    replica_groups=[list(range(cfg.n_cores))],
                    ins=[ag_in[:]], outs=[xl_full[:]])
                edge_phase(li, xl_full, xr_sh)

            # final linear
            kcs = _chunks(cfg.f_final, P)
            for m in range(NB):
                mo = m * P
                mn = min(P, SH - mo)
                pf = ps2.tile([P, 1], F32, name="pf", tag="small_ps", bufs=2)
                lhs = []
                for ki, (ko, ks) in enumerate(kcs):
                    lt = sb.tile([ks, P], F16, name="lhsTf", tag=f"lhsTf{ki}")
                    nc.sync.dma_start(out=lt[:, :mn], in_=hT_dram[-1][ko:ko + ks, mo:mo + mn])
                    lhs.append(lt)
                for ki, (ko, ks) in enumerate(kcs):
                    nc.tensor.matmul(out=pf[:mn, :], lhsT=lhs[ki][:, :mn],
                                     rhs=wf_sb[:ks, ki:ki + 1],
                                     start=(ki == 0), stop=(ki == len(kcs) - 1))
                of = sb.tile([P, 1], F32, name="of", tag="of")
                nc.scalar.activation(out=of[:mn, :], in_=pf[:mn, :],
                                     func=mybir.ActivationFunctionType.Identity,
                                     bias=bf_sb[:mn, :], scale=1.0)
                nc.sync.dma_start(out=out[mo:mo + mn, :], in_=of[:mn, :])

    nc.compile()
    return nc


# =====================================================================
# host-side data prep
# =====================================================================

def prep_host(inputs, cfg: GatCfg):
    N, SH, NB = cfg.n_nodes, cfg.shard, cfg.nblk
    x = np.asarray(inputs['x'], dtype=np.float32)
    ei = np.asarray(inputs['edge_index']).astype(np.int64)
    loop = np.arange(N, dtype=np.int64)
    src = np.concatenate([ei[0], loop])
    dst = np.concatenate([ei[1], loop])
    order = np.argsort(dst, kind='stable')
    src_s, dst_s = src[order], dst[order]

    cnt = np.zeros((cfg.n_cores, NB), dtype=np.int64)
    bounds = {}
    for c in range(cfg.n_cores):
        for b in range(NB):
            blk_lo = c * SH + b * P
            blk_hi = min(blk_lo + P, (c + 1) * SH)
            lo = np.searchsorted(dst_s, blk_lo)
            hi = np.searchsorted(dst_s, blk_hi)
            bounds[(c, b)] = (lo, hi, blk_lo)
            cnt[c, b] = hi - lo
    T = int((cnt.max() + P - 1) // P)
    cfg.T = T

    iarange = np.arange(P, dtype=np.int64)
    in_maps = []
    for c in range(cfg.n_cores):
        srcs = np.zeros((NB, P, T), dtype=np.int32)
        oh16 = np.zeros((NB, P, T * P), dtype=np.float16)
        de16 = np.zeros((NB, P, T * P), dtype=np.float16)
        for b in range(NB):
            lo, hi, blk_lo = bounds[(c, b)]
            ne = hi - lo
            s = np.zeros(T * P, dtype=np.int32)
            d = np.full(T * P, -1, dtype=np.int64)
            s[:ne] = src_s[lo:hi]
            d[:ne] = (dst_s[lo:hi] - blk_lo)
            srcs[b] = s.reshape(T, P).T
            oh_lin = (d[:, None] == iarange[None, :]).astype(np.float16)  # [T*P, P]
            oh_r = oh_lin.reshape(T, P, P)
            oh16[b] = oh_r.transpose(1, 0, 2).reshape(P, T * P)
            de16[b] = oh_r.transpose(2, 0, 1).reshape(P, T * P)
        xT = np.ascontiguousarray(x[c * SH:(c + 1) * SH, :].T).astype(np.float16)
        im = {'srcs': srcs, 'oh16': oh16, 'de16': de16, 'xT': xT}
        for li, L in enumerate(cfg.layers):
            Wl = np.asarray(inputs[f'Wl{li + 1}'], np.float32)
            Wr = np.asarray(inputs[f'Wr{li + 1}'], np.float32)
            att = np.asarray(inputs[f'att{li + 1}'], np.float32).reshape(1, -1)
            bias = np.asarray(inputs[f'b{li + 1}'], np.float32).reshape(-1)
            nkc = len(_chunks(L.hc, P))
            bT = np.zeros((P, nkc), dtype=np.float32)
            for kc, (fo, fs) in enumerate(_chunks(L.hc, P)):
                bT[:fs, kc] = bias[fo:fo + fs]
            att_bd = np.zeros((L.hc, L.heads), dtype=np.float32)
            for h in range(L.heads):
                att_bd[h * L.out_ch:(h + 1) * L.out_ch, h] = \
                    np.asarray(inputs[f'att{li + 1}'], np.float32)[h]
            wl_aug = np.concatenate([Wl, 0.6 * (Wl @ att_bd)], axis=1)
            wr_aug = np.concatenate([Wr, 0.6 * (Wr @ att_bd)], axis=1)
            im[f'wl{li}'] = wl_aug.astype(np.float16)
            im[f'wr{li}'] = wr_aug.astype(np.float16)
            im[f'att{li}'] = np.repeat(att * 0.4, P, axis=0).astype(np.float16)
            im[f'biasT{li}'] = bT
        wf_flat = np.asarray(inputs['Wf'], np.float32).reshape(-1)
        nkf = len(_chunks(cfg.f_final, P))
        wfp = np.zeros((P, nkf), dtype=np.float32)
        for ki, (ko, ks) in enumerate(_chunks(cfg.f_final, P)):
            wfp[:ks, ki] = wf_flat[ko:ko + ks]
        im['wf'] = wfp.astype(np.float16)
        im['bf_col'] = np.full((P, 1), np.asarray(inputs['bf'], np.float32).reshape(-1)[0],
                               dtype=np.float32)
        in_maps.append(im)
    return in_maps, T


_CACHE = {}


def kernel(**inputs) -> np.ndarray:
    cfg = GatCfg()
    in_maps, T = prep_host(inputs, cfg)
    key = ('v2', T)
    if key not in _CACHE:
        _CACHE[key] = build_gat(cfg)
    nc = _CACHE[key]
    res = bass_utils.run_bass_kernel_spmd(nc, in_maps, core_ids=list(range(cfg.n_cores)))
    out = np.concatenate([res.results[c]['out'] for c in range(cfg.n_cores)], axis=0)
    return out.astype(np.float32)


# revision 4
# speedup vs baseline: 1.3713x; 1.0074x over previous
"""GATv2 (3-layer) Trainium2 Bass kernel, 8-core SPMD — v2.

Strategy
--------
- Nodes sharded 2500/core; edges (incl. self-loops) sorted by dst and
  sharded by dst range, so segment-softmax + aggregation are core-local.
- Per layer:
    GEMM phase (data parallel over own nodes): XL = H @ Wl_aug,
      XR = H @ Wr_aug (aug cols carry 0.6*att.s linear-logit terms).
    AllGather XL shards -> XL_full [20000, HCA] (for src gathers).
    Edge phase: dst blocks of 128 nodes x T edge tiles of 128 edges.
      One-hot matrices oh16 (edge-major) / de16 (dst-major) are STATIC
      per graph — built on host, streamed from DRAM per block.
      Per tile:
        indirect-gather xl[src]; sp = de16 @ xr + I @ xl (PSUM);
        lr = |sp| (ACT Abs); prod = lr * att (DVE f16);
        red = per-head sum (DVE reduce); logit = red + sp_aug (DVE);
        w = exp(logit - SHIFT) (ACT); rsc = [w (.) xl | w16];
        u_ps[:, 0:HC+H] += oh16^T @ rsc  (accumulates both the
          unnormalized weighted sum AND the softmax denominator D).
      Epilogue per block: recip(D); u16 = u_ps * recip_h (ACT, PSUM
      read); transpose chunks (PE, f16); Relu(u + bias) -> hT (DRAM).
- Final: out = H3 @ Wf + bf, node-sharded; host concatenates 8 shards.

Numerics: features/weights fp16, PSUM fp32, logits fp32, exp shifted
by -4 so w = exp(logit-4) <= ~4 stays in fp16 range. alpha = w/D is
applied after aggregation (exactly equal to normalizing per edge).
"""
import sys
if '/opt/trn_rl_repo' not in sys.path:
    sys.path.insert(0, '/opt/trn_rl_repo')

from dataclasses import dataclass
import numpy as np

import concourse.bass as bass
import concourse.bacc as bacc
import concourse.tile as tile
from concourse import mybir
from concourse import bass_utils
from concourse.masks import make_identity

P = 128
F32 = mybir.dt.float32
F16 = mybir.dt.float16
I32 = mybir.dt.int32

EXP_SHIFT = 4.0  # logits measured in [-6.1, 5.4]; any constant is exact math-wise


@dataclass
class LayerCfg:
    f_in: int
    heads: int
    out_ch: int

    @property
    def hc(self):
        return self.heads * self.out_ch

    @property
    def hca(self):
        return self.heads * self.out_ch + self.heads


@dataclass
class GatCfg:
    n_cores: int = 8
    shard: int = 2500          # nodes per core
    T: int = 18                # edge tiles per dst block
    layers: tuple = (LayerCfg(64, 3, 64), LayerCfg(192, 3, 256), LayerCfg(768, 1, 512))
    f_final: int = 512
    bufs_sp: int = 2           # PSUM bufs for sp tiles
    bufs_xl: int = 4           # gather destination bufs
    bufs_sb: int = 2
    bufs_blk: int = 2

    @property
    def n_nodes(self):
        return self.n_cores * self.shard

    @property
    def nblk(self):
        return (self.shard + P - 1) // P

    @property
    def hc_max(self):
        return max(L.hc for L in self.layers)


def _chunks(total, step):
    out = []
    off = 0
    while off < total:
        sz = min(step, total - off)
        out.append((off, sz))
        off += sz
    return out


def build_gat(cfg: GatCfg):
    nc = bacc.Bacc("TRN2", target_bir_lowering=False, debug=False,
                   num_devices=cfg.n_cores)
    NB, T, SH = cfg.nblk, cfg.T, cfg.shard

    # ---------------- external tensors (per-core) ----------------
    srcs = nc.dram_tensor("srcs", [NB, P, T], I32, kind="ExternalInput").ap()
    oh16_d = nc.dram_tensor("oh16", [NB, P, T * P], F16, kind="ExternalInput").ap()
    de16_d = nc.dram_tensor("de16", [NB, P, T * P], F16, kind="ExternalInput").ap()
    xT = nc.dram_tensor("xT", [cfg.layers[0].f_in, SH], F16, kind="ExternalInput").ap()

    wl_d, wr_d, att_d, biasT_d = [], [], [], []
    for li, L in enumerate(cfg.layers):
        nkc = len(_chunks(L.hc, P))
        wl_d.append(nc.dram_tensor(f"wl{li}", [L.f_in, L.hca], F16, kind="ExternalInput").ap())
        wr_d.append(nc.dram_tensor(f"wr{li}", [L.f_in, L.hca], F16, kind="ExternalInput").ap())
        att_d.append(nc.dram_tensor(f"att{li}", [P, L.hc], F16, kind="ExternalInput").ap())
        biasT_d.append(nc.dram_tensor(f"biasT{li}", [P, nkc], F32, kind="ExternalInput").ap())
    nkf = len(_chunks(cfg.f_final, P))
    wf = nc.dram_tensor("wf", [P, nkf], F16, kind="ExternalInput").ap()
    bf_col = nc.dram_tensor("bf_col", [P, 1], F32, kind="ExternalInput").ap()

    out = nc.dram_tensor("out", [SH, 1], F32, kind="ExternalOutput").ap()

    with tile.TileContext(nc) as tc:
        with tc.tile_pool(name="const", bufs=1) as constp, \
             tc.tile_pool(name="wpool", bufs=1) as wpool, \
             tc.tile_pool(name="sb", bufs=cfg.bufs_sb) as sb, \
             tc.tile_pool(name="blk", bufs=cfg.bufs_blk) as blk, \
             tc.tile_pool(name="ps", bufs=1, space="PSUM") as ps, \
             tc.tile_pool(name="ps2", bufs=2, space="PSUM") as ps2, \
             tc.tile_pool(name="dram", bufs=1, space="DRAM") as dram:

            # ---------------- constants ----------------
            ident32 = constp.tile([P, P], F32, name="ident32")
            make_identity(nc, ident32[:])
            ident16 = constp.tile([P, P], F16, name="ident16")
            nc.vector.tensor_copy(out=ident16[:], in_=ident32[:])
            shift_col = constp.tile([P, 1], F32, name="shift_col")
            nc.vector.memset(shift_col[:], -EXP_SHIFT)

            # resident weights / att / biasT
            wl_sb, wr_sb, att_sb, biasT_sb = [], [], [], []
            for li, L in enumerate(cfg.layers):
                wlk, wrk = [], []
                for ki, (ko, ks) in enumerate(_chunks(L.f_in, P)):
                    t1 = wpool.tile([ks, L.hca], F16, name=f"wl{li}k{ki}")
                    nc.sync.dma_start(out=t1[:], in_=wl_d[li][ko:ko + ks, :])
                    wlk.append(t1)
                    t2 = wpool.tile([ks, L.hca], F16, name=f"wr{li}k{ki}")
                    nc.sync.dma_start(out=t2[:], in_=wr_d[li][ko:ko + ks, :])
                    wrk.append(t2)
                wl_sb.append(wlk)
                wr_sb.append(wrk)
                ta = wpool.tile([P, L.hc], F16, name=f"att{li}")
                nc.sync.dma_start(out=ta[:], in_=att_d[li][:])
                att_sb.append(ta)
                nkc = len(_chunks(L.hc, P))
                tb = wpool.tile([P, nkc], F32, name=f"biasT{li}")
                nc.sync.dma_start(out=tb[:], in_=biasT_d[li][:])
                biasT_sb.append(tb)
            wf_sb = wpool.tile([P, nkf], F16, name="wf_sb")
            nc.sync.dma_start(out=wf_sb[:], in_=wf[:])
            bf_sb = wpool.tile([P, 1], F32, name="bf_sb")
            nc.sync.dma_start(out=bf_sb[:], in_=bf_col[:])

            hT_dram = []

            # =========================================================
            def gemm_phase(li):
                L = cfg.layers[li]
                kcs = _chunks(L.f_in, P)
                ncs = _chunks(L.hca, 512)
                ag_in = dram.tile([SH, L.hca], F16, name=f"ag_in{li}")
                xr_sh = dram.tile([NB * P, L.hca], F16, name=f"xr{li}")
                pad = NB * P - SH
                if pad:
                    ztile = sb.tile([pad, L.hca], F16, name="zpad", tag="zpad", bufs=1)
                    nc.vector.memset(ztile[:], 0.0)
                    nc.sync.dma_start(out=xr_sh[SH:NB * P, :], in_=ztile[:])
                src_ap = xT if li == 0 else hT_dram[li - 1]
                for m in range(NB):
                    mo = m * P
                    mn = min(P, SH - mo)
                    lhs = []
                    for ki, (ko, ks) in enumerate(kcs):
                        lt = sb.tile([ks, P], F16, name="lhsT", tag=f"lhsT{ki}")
                        nc.sync.dma_start(out=lt[:, :mn], in_=src_ap[ko:ko + ks, mo:mo + mn])
                        lhs.append(lt)
                    for wsb, dst_d in ((wl_sb[li], ag_in), (wr_sb[li], xr_sh)):
                        og = sb.tile([P, L.hca], F16, name="og", tag="og")
                        for (no, ns) in ncs:
                            pg = ps2.tile([P, ns], F32, name="pg", tag="mm512", bufs=2)
                            for ki in range(len(kcs)):
                                nc.tensor.matmul(
                                    out=pg[:mn, :ns],
                                    lhsT=lhs[ki][:, :mn],
                                    rhs=wsb[ki][:, no:no + ns],
                                    start=(ki == 0), stop=(ki == len(kcs) - 1))
                            nc.scalar.copy(out=og[:mn, no:no + ns], in_=pg[:mn, :ns])
                        nc.sync.dma_start(out=dst_d[mo:mo + mn, :], in_=og[:mn, :])
                return ag_in, xr_sh

            # =========================================================
            def edge_phase(li, xl_full, xr_sh):
                L = cfg.layers[li]
                H, C, HC, HCA = L.heads, L.out_ch, L.hc, L.hca
                ncs = _chunks(HCA, 512)
                ucs = _chunks(HC + H, 512)
                for b in range(NB):
                    bn = min(P, SH - b * P)
                    src_i = blk.tile([P, T], I32, name="src_i", tag="src_i")
                    nc.sync.dma_start(out=src_i[:], in_=srcs[b, :, :])
                    oh_b = blk.tile([P, T * P], F16, name="oh_b", tag="oh_b")
                    nc.sync.dma_start(out=oh_b[:], in_=oh16_d[b, :, :])
                    de_b = blk.tile([P, T * P], F16, name="de_b", tag="de_b")
                    nc.sync.dma_start(out=de_b[:], in_=de16_d[b, :, :])
                    xr_blk = blk.tile([P, HCA], F16, name="xr_blk", tag="xr_blk")
                    nc.sync.dma_start(out=xr_blk[:], in_=xr_sh[b * P:(b + 1) * P, :])

                    u_ps = ps.tile([P, HC + H], F32, name="u_ps", tag="u_ps")

                    for t in range(T):
                        xl_g = sb.tile([P, HCA], F16, name="xl_g", tag="xl_g",
                                       bufs=cfg.bufs_xl)
                        nc.gpsimd.indirect_dma_start(
                            out=xl_g[:], out_offset=None, in_=xl_full[:],
                            in_offset=bass.IndirectOffsetOnAxis(ap=src_i[:, t:t + 1], axis=0))
                        sp = ps2.tile([P, HCA], F32, name="sp", tag="mm512",
                                      bufs=cfg.bufs_sp)
                        for (no, ns) in ncs:
                            nc.tensor.matmul(out=sp[:, no:no + ns],
                                             lhsT=de_b[:, t * P:(t + 1) * P],
                                             rhs=xr_blk[:, no:no + ns],
                                             start=True, stop=False)
                            nc.tensor.matmul(out=sp[:, no:no + ns],
                                             lhsT=ident16[:],
                                             rhs=xl_g[:, no:no + ns],
                                             start=False, stop=True)
                        lr = sb.tile([P, HC], F16, name="lr", tag="lr")
                        nc.scalar.activation(out=lr[:], in_=sp[:, :HC],
                                             func=mybir.ActivationFunctionType.Abs)
                        prod = sb.tile([P, HC], F16, name="prod", tag="prod")
                        nc.vector.tensor_tensor(out=prod[:], in0=lr[:],
                                                in1=att_sb[li][:],
                                                op=mybir.AluOpType.mult)
                        red = sb.tile([P, H], F32, name="red", tag="red")
                        nc.vector.tensor_reduce(
                            out=red[:], in_=prod[:].rearrange("p (h c) -> p h c", h=H),
                            axis=mybir.AxisListType.X, op=mybir.AluOpType.add)
                        logit = sb.tile([P, H], F32, name="logit", tag="logit")
                        nc.vector.tensor_tensor(out=logit[:], in0=red[:],
                                                in1=sp[:, HC:HC + H],
                                                op=mybir.AluOpType.add)
                        w32 = sb.tile([P, H], F32, name="w32", tag="w32")
                        nc.scalar.activation(out=w32[:], in_=logit[:],
                                             func=mybir.ActivationFunctionType.Exp,
                                             bias=shift_col[:])
                        rsc = sb.tile([P, HC + H], F16, name="rsc", tag="rsc", bufs=3)
                        for h in range(H):
                            nc.vector.tensor_scalar(
                                out=rsc[:, h * C:(h + 1) * C], in0=xl_g[:, h * C:(h + 1) * C],
                                scalar1=w32[:, h:h + 1], scalar2=None,
                                op0=mybir.AluOpType.mult)
                        nc.vector.tensor_copy(out=rsc[:, HC:HC + H], in_=w32[:])
                        for (no, ns) in ucs:
                            nc.tensor.matmul(out=u_ps[:, no:no + ns],
                                             lhsT=oh_b[:, t * P:(t + 1) * P],
                                             rhs=rsc[:, no:no + ns],
                                             start=(t == 0), stop=(t == T - 1))

                    # ---------------- epilogue ----------------
                    dsb = sb.tile([P, H], F32, name="dsb", tag="dsb")
                    nc.vector.tensor_scalar(out=dsb[:], in0=u_ps[:, HC:HC + H],
                                            scalar1=1e-30, scalar2=None,
                                            op0=mybir.AluOpType.add)
                    recip = sb.tile([P, H], F32, name="recip", tag="recip")
                    nc.vector.reciprocal(out=recip[:], in_=dsb[:])
                    u16 = sb.tile([P, HC], F16, name="u16", tag="u16")
                    for h in range(H):
                        nc.scalar.activation(
                            out=u16[:, h * C:(h + 1) * C], in_=u_ps[:, h * C:(h + 1) * C],
                            func=mybir.ActivationFunctionType.Copy,
                            scale=recip[:, h:h + 1])
                    for kc, (fo, fs) in enumerate(_chunks(HC, P)):
                        tp2 = ps2.tile([P, P], F16, name="tp2", tag="small_ps", bufs=2)
                        nc.tensor.transpose(out=tp2[:fs, :], in_=u16[:, fo:fo + fs],
                                            identity=ident16[:])
                        hts = sb.tile([P, P], F16, name="hts", tag="hts")
                        nc.scalar.activation(out=hts[:fs, :bn], in_=tp2[:fs, :bn],
                                             func=mybir.ActivationFunctionType.Relu,
                                             bias=biasT_sb[li][:fs, kc:kc + 1],
                                             scale=1.0)
                        nc.sync.dma_start(
                            out=hT_dram[li][fo:fo + fs, b * P:b * P + bn],
                            in_=hts[:fs, :bn])

            # =========================================================
            for li, L in enumerate(cfg.layers):
                hT_dram.append(dram.tile([L.hc, SH], F16, name=f"hT{li}"))
            for li, L in enumerate(cfg.layers):
                ag_in, xr_sh = gemm_phase(li)
                xl_full = dram.tile([cfg.n_nodes, L.hca], F16, name=f"xl_full{li}",
                                    addr_space="Shared")
                nc.gpsimd.collective_compute(
                    "AllGather", mybir.AluOpType.bypass,
                    replica_groups=[list(range(cfg.n_cores))],
                    ins=[ag_in[:]], outs=[xl_full[:]])
                edge_phase(li, xl_full, xr_sh)

            # final linear
            kcs = _chunks(cfg.f_final, P)
            for m in range(NB):
                mo = m * P
                mn = min(P, SH - mo)
                pf = ps2.tile([P, 1], F32, name="pf", tag="small_ps", bufs=2)
                lhs = []
                for ki, (ko, ks) in enumerate(kcs):
                    lt = sb.tile([ks, P], F16, name="lhsTf", tag=f"lhsTf{ki}")
                    nc.sync.dma_start(out=lt[:, :mn], in_=hT_dram[-1][ko:ko + ks, mo:mo + mn])
                    lhs.append(lt)
                for ki, (ko, ks) in enumerate(kcs):
                    nc.tensor.matmul(out=pf[:mn, :], lhsT=lhs[ki][:, :mn],
                                     rhs=wf_sb[:ks, ki:ki + 1],
                                     start=(ki == 0), stop=(ki == len(kcs) - 1))
                of = sb.tile([P, 1], F32, name="of", tag="of")
                nc.scalar.activation(out=of[:mn, :], in_=pf[:mn, :],
                                     func=mybir.ActivationFunctionType.Identity,
                                     bias=bf_sb[:mn, :], scale=1.0)
                nc.sync.dma_start(out=out[mo:mo + mn, :], in_=of[:mn, :])

    nc.compile()
    return nc


# =====================================================================
# host-side data prep
# =====================================================================

def prep_host(inputs, cfg: GatCfg):
    N, SH, NB = cfg.n_nodes, cfg.shard, cfg.nblk
    x = np.asarray(inputs['x'], dtype=np.float32)
    ei = np.asarray(inputs['edge_index']).astype(np.int64)
    loop = np.arange(N, dtype=np.int64)
    src = np.concatenate([ei[0], loop])
    dst = np.concatenate([ei[1], loop])
    order = np.argsort(dst, kind='stable')
    src_s, dst_s = src[order], dst[order]

    cnt = np.zeros((cfg.n_cores, NB), dtype=np.int64)
    bounds = {}
    for c in range(cfg.n_cores):
        for b in range(NB):
            blk_lo = c * SH + b * P
            blk_hi = min(blk_lo + P, (c + 1) * SH)
            lo = np.searchsorted(dst_s, blk_lo)
            hi = np.searchsorted(dst_s, blk_hi)
            bounds[(c, b)] = (lo, hi, blk_lo)
            cnt[c, b] = hi - lo
    T = int((cnt.max() + P - 1) // P)
    cfg.T = T

    iarange = np.arange(P, dtype=np.int64)
    in_maps = []
    for c in range(cfg.n_cores):
        srcs = np.zeros((NB, P, T), dtype=np.int32)
        oh16 = np.zeros((NB, P, T * P), dtype=np.float16)
        de16 = np.zeros((NB, P, T * P), dtype=np.float16)
        for b in range(NB):
            lo, hi, blk_lo = bounds[(c, b)]
            ne = hi - lo
            s = np.zeros(T * P, dtype=np.int32)
            d = np.full(T * P, -1, dtype=np.int64)
            s[:ne] = src_s[lo:hi]
            d[:ne] = (dst_s[lo:hi] - blk_lo)
            srcs[b] = s.reshape(T, P).T
            oh_lin = (d[:, None] == iarange[None, :]).astype(np.float16)  # [T*P, P]
            oh_r = oh_lin.reshape(T, P, P)
            oh16[b] = oh_r.transpose(1, 0, 2).reshape(P, T * P)
            de16[b] = oh_r.transpose(2, 0, 1).reshape(P, T * P)
        xT = np.ascontiguousarray(x[c * SH:(c + 1) * SH, :].T).astype(np.float16)
        im = {'srcs': srcs, 'oh16': oh16, 'de16': de16, 'xT': xT}
        for li, L in enumerate(cfg.layers):
            Wl = np.asarray(inputs[f'Wl{li + 1}'], np.float32)
            Wr = np.asarray(inputs[f'Wr{li + 1}'], np.float32)
            att = np.asarray(inputs[f'att{li + 1}'], np.float32).reshape(1, -1)
            bias = np.asarray(inputs[f'b{li + 1}'], np.float32).reshape(-1)
            nkc = len(_chunks(L.hc, P))
            bT = np.zeros((P, nkc), dtype=np.float32)
            for kc, (fo, fs) in enumerate(_chunks(L.hc, P)):
                bT[:fs, kc] = bias[fo:fo + fs]
            att_bd = np.zeros((L.hc, L.heads), dtype=np.float32)
            for h in range(L.heads):
                att_bd[h * L.out_ch:(h + 1) * L.out_ch, h] = \
                    np.asarray(inputs[f'att{li + 1}'], np.float32)[h]
            wl_aug = np.concatenate([Wl, 0.6 * (Wl @ att_bd)], axis=1)
            wr_aug = np.concatenate([Wr, 0.6 * (Wr @ att_bd)], axis=1)
            im[f'wl{li}'] = wl_aug.astype(np.float16)
            im[f'wr{li}'] = wr_aug.astype(np.float16)
            im[f'att{li}'] = np.repeat(att * 0.4, P, axis=0).astype(np.float16)
            im[f'biasT{li}'] = bT
        wf_flat = np.asarray(inputs['Wf'], np.float32).reshape(-1)
        nkf = len(_chunks(cfg.f_final, P))
        wfp = np.zeros((P, nkf), dtype=np.float32)
        for ki, (ko, ks) in enumerate(_chunks(cfg.f_final, P)):
            wfp[:ks, ki] = wf_flat[ko:ko + ks]
        im['wf'] = wfp.astype(np.float16)
        im['bf_col'] = np.full((P, 1), np.asarray(inputs['bf'], np.float32).reshape(-1)[0],
                               dtype=np.float32)
        in_maps.append(im)
    return in_maps, T


_CACHE = {}


def kernel(**inputs) -> np.ndarray:
    cfg = GatCfg()
    in_maps, T = prep_host(inputs, cfg)
    key = ('v2', T)
    if key not in _CACHE:
        _CACHE[key] = build_gat(cfg)
    nc = _CACHE[key]
    res = bass_utils.run_bass_kernel_spmd(nc, in_maps, core_ids=list(range(cfg.n_cores)))
    out = np.concatenate([res.results[c]['out'] for c in range(cfg.n_cores)], axis=0)
    return out.astype(np.float32)


# revision 6
# speedup vs baseline: 1.5894x; 1.1590x over previous
"""GATv2 (3-layer) Trainium2 Bass kernel, 8-core SPMD — v2.

Strategy
--------
- Nodes sharded 2500/core; edges (incl. self-loops) sorted by dst and
  sharded by dst range, so segment-softmax + aggregation are core-local.
- Per layer:
    GEMM phase (data parallel over own nodes): XL = H @ Wl_aug,
      XR = H @ Wr_aug (aug cols carry 0.6*att.s linear-logit terms).
    AllGather XL shards -> XL_full [20000, HCA] (for src gathers).
    Edge phase: dst blocks of 128 nodes x T edge tiles of 128 edges.
      One-hot matrices oh16 (edge-major) / de16 (dst-major) are STATIC
      per graph — built on host, streamed from DRAM per block.
      Per tile:
        indirect-gather xl[src]; sp = de16 @ xr + I @ xl (PSUM);
        lr = |sp| (ACT Abs); prod = lr * att (DVE f16);
        red = per-head sum (DVE reduce); logit = red + sp_aug (DVE);
        w = exp(logit - SHIFT) (ACT); rsc = [w (.) xl | w16];
        u_ps[:, 0:HC+H] += oh16^T @ rsc  (accumulates both the
          unnormalized weighted sum AND the softmax denominator D).
      Epilogue per block: recip(D); u16 = u_ps * recip_h (ACT, PSUM
      read); transpose chunks (PE, f16); Relu(u + bias) -> hT (DRAM).
- Final: out = H3 @ Wf + bf, node-sharded; host concatenates 8 shards.

Numerics: features/weights fp16, PSUM fp32, logits fp32, exp shifted
by -4 so w = exp(logit-4) <= ~4 stays in fp16 range. alpha = w/D is
applied after aggregation (exactly equal to normalizing per edge).
"""
import sys
if '/opt/trn_rl_repo' not in sys.path:
    sys.path.insert(0, '/opt/trn_rl_repo')

from dataclasses import dataclass
import numpy as np

import concourse.bass as bass
import concourse.bacc as bacc
import concourse.tile as tile
from concourse import mybir
from concourse import bass_utils
from concourse.masks import make_identity

P = 128
F32 = mybir.dt.float32
F16 = mybir.dt.float16
I32 = mybir.dt.int32

EXP_SHIFT = 4.0  # logits measured in [-6.1, 5.4]; any constant is exact math-wise


@dataclass
class LayerCfg:
    f_in: int
    heads: int
    out_ch: int

    @property
    def hc(self):
        return self.heads * self.out_ch

    @property
    def hca(self):
        return self.heads * self.out_ch + self.heads


@dataclass
class GatCfg:
    n_cores: int = 8
    shard: int = 2500          # nodes per core
    T: int = 18                # edge tiles per dst block
    layers: tuple = (LayerCfg(64, 3, 64), LayerCfg(192, 3, 256), LayerCfg(768, 1, 512))
    f_final: int = 512
    bufs_sp: int = 4           # PSUM bufs for sp chunk tiles
    bufs_xl: int = 8           # gather destination bufs
    bufs_sb: int = 2
    bufs_blk: int = 3

    @property
    def n_nodes(self):
        return self.n_cores * self.shard

    @property
    def nblk(self):
        return (self.shard + P - 1) // P

    @property
    def hc_max(self):
        return max(L.hc for L in self.layers)


def _chunks(total, step):
    out = []
    off = 0
    while off < total:
        sz = min(step, total - off)
        out.append((off, sz))
        off += sz
    return out


def build_gat(cfg: GatCfg):
    nc = bacc.Bacc("TRN2", target_bir_lowering=False, debug=False,
                   num_devices=cfg.n_cores)
    NB, T, SH = cfg.nblk, cfg.T, cfg.shard

    # ---------------- external tensors (per-core) ----------------
    srcs = nc.dram_tensor("srcs", [NB, P, T], I32, kind="ExternalInput").ap()
    oh16_d = nc.dram_tensor("oh16", [NB, P, T * P], F16, kind="ExternalInput").ap()
    de16_d = nc.dram_tensor("de16", [NB, P, T * P], F16, kind="ExternalInput").ap()
    xT = nc.dram_tensor("xT", [cfg.layers[0].f_in, SH], F16, kind="ExternalInput").ap()

    wl_d, wr_d, att_d, biasT_d = [], [], [], []
    for li, L in enumerate(cfg.layers):
        nkc = len(_chunks(L.hc, P))
        wl_d.append(nc.dram_tensor(f"wl{li}", [L.f_in, L.hca], F16, kind="ExternalInput").ap())
        wr_d.append(nc.dram_tensor(f"wr{li}", [L.f_in, L.hca], F16, kind="ExternalInput").ap())
        att_d.append(nc.dram_tensor(f"att{li}", [P, L.hc], F16, kind="ExternalInput").ap())
        biasT_d.append(nc.dram_tensor(f"biasT{li}", [P, nkc], F32, kind="ExternalInput").ap())
    nkf = len(_chunks(cfg.f_final, P))
    wf = nc.dram_tensor("wf", [P, nkf], F16, kind="ExternalInput").ap()
    bf_col = nc.dram_tensor("bf_col", [P, 1], F32, kind="ExternalInput").ap()

    out = nc.dram_tensor("out", [SH, 1], F32, kind="ExternalOutput").ap()

    with tile.TileContext(nc) as tc:
        with tc.tile_pool(name="const", bufs=1) as constp, \
             tc.tile_pool(name="wpool", bufs=1) as wpool, \
             tc.tile_pool(name="sb", bufs=cfg.bufs_sb) as sb, \
             tc.tile_pool(name="blk", bufs=cfg.bufs_blk) as blk, \
             tc.tile_pool(name="ps", bufs=1, space="PSUM") as ps, \
             tc.tile_pool(name="ps2", bufs=2, space="PSUM") as ps2, \
             tc.tile_pool(name="dram", bufs=1, space="DRAM") as dram:

            # ---------------- constants ----------------
            ident32 = constp.tile([P, P], F32, name="ident32")
            make_identity(nc, ident32[:])
            ident16 = constp.tile([P, P], F16, name="ident16")
            nc.vector.tensor_copy(out=ident16[:], in_=ident32[:])
            shift_col = constp.tile([P, 1], F32, name="shift_col")
            nc.vector.memset(shift_col[:], -EXP_SHIFT)

            # resident weights / att / biasT
            wl_sb, wr_sb, att_sb, biasT_sb = [], [], [], []
            for li, L in enumerate(cfg.layers):
                wlk, wrk = [], []
                for ki, (ko, ks) in enumerate(_chunks(L.f_in, P)):
                    t1 = wpool.tile([ks, L.hca], F16, name=f"wl{li}k{ki}")
                    nc.sync.dma_start(out=t1[:], in_=wl_d[li][ko:ko + ks, :])
                    wlk.append(t1)
                    t2 = wpool.tile([ks, L.hca], F16, name=f"wr{li}k{ki}")
                    nc.sync.dma_start(out=t2[:], in_=wr_d[li][ko:ko + ks, :])
                    wrk.append(t2)
                wl_sb.append(wlk)
                wr_sb.append(wrk)
                ta = wpool.tile([P, L.hc], F16, name=f"att{li}")
                nc.sync.dma_start(out=ta[:], in_=att_d[li][:])
                att_sb.append(ta)
                nkc = len(_chunks(L.hc, P))
                tb = wpool.tile([P, nkc], F32, name=f"biasT{li}")
                nc.sync.dma_start(out=tb[:], in_=biasT_d[li][:])
                biasT_sb.append(tb)
            wf_sb = wpool.tile([P, nkf], F16, name="wf_sb")
            nc.sync.dma_start(out=wf_sb[:], in_=wf[:])
            bf_sb = wpool.tile([P, 1], F32, name="bf_sb")
            nc.sync.dma_start(out=bf_sb[:], in_=bf_col[:])

            hT_dram = []

            # =========================================================
            def gemm_phase(li):
                L = cfg.layers[li]
                kcs = _chunks(L.f_in, P)
                ncs = _chunks(L.hca, 512)
                ag_in = dram.tile([SH, L.hca], F16, name=f"ag_in{li}")
                xr_sh = dram.tile([NB * P, L.hca], F16, name=f"xr{li}")
                pad = NB * P - SH
                if pad:
                    ztile = sb.tile([pad, L.hca], F16, name="zpad", tag="zpad", bufs=1)
                    nc.vector.memset(ztile[:], 0.0)
                    nc.sync.dma_start(out=xr_sh[SH:NB * P, :], in_=ztile[:])
                src_ap = xT if li == 0 else hT_dram[li - 1]
                for m in range(NB):
                    mo = m * P
                    mn = min(P, SH - mo)
                    lhs = []
                    for ki, (ko, ks) in enumerate(kcs):
                        lt = sb.tile([ks, P], F16, name="lhsT", tag=f"lhsT{ki}")
                        nc.sync.dma_start(out=lt[:, :mn], in_=src_ap[ko:ko + ks, mo:mo + mn])
                        lhs.append(lt)
                    for wsb, dst_d in ((wl_sb[li], ag_in), (wr_sb[li], xr_sh)):
                        og = sb.tile([P, L.hca], F16, name="og", tag="og")
                        for (no, ns) in ncs:
                            pg = ps2.tile([P, ns], F32, name="pg", tag="mm512", bufs=cfg.bufs_sp)
                            for ki in range(len(kcs)):
                                nc.tensor.matmul(
                                    out=pg[:mn, :ns],
                                    lhsT=lhs[ki][:, :mn],
                                    rhs=wsb[ki][:, no:no + ns],
                                    start=(ki == 0), stop=(ki == len(kcs) - 1))
                            nc.scalar.copy(out=og[:mn, no:no + ns], in_=pg[:mn, :ns])
                        nc.sync.dma_start(out=dst_d[mo:mo + mn, :], in_=og[:mn, :])
                return ag_in, xr_sh

            # =========================================================
            def edge_phase(li, xl_full, xr_sh):
                L = cfg.layers[li]
                H, C, HC, HCA = L.heads, L.out_ch, L.hc, L.hca
                ncs = _chunks(HCA, 512)
                ucs = _chunks(HC + H, 512)
                for b in range(NB):
                    bn = min(P, SH - b * P)
                    src_i = blk.tile([P, T], I32, name="src_i", tag="src_i")
                    nc.sync.dma_start(out=src_i[:], in_=srcs[b, :, :])
                    oh_b = blk.tile([P, T * P], F16, name="oh_b", tag="oh_b")
                    nc.sync.dma_start(out=oh_b[:], in_=oh16_d[b, :, :])
                    de_b = blk.tile([P, T * P], F16, name="de_b", tag="de_b")
                    nc.sync.dma_start(out=de_b[:], in_=de16_d[b, :, :])
                    xr_blk = blk.tile([P, HCA], F16, name="xr_blk", tag="xr_blk")
                    nc.sync.dma_start(out=xr_blk[:], in_=xr_sh[b * P:(b + 1) * P, :])

                    u_ps = ps.tile([P, HC + H], F32, name="u_ps", tag="u_ps")

                    for t in range(T):
                        xl_g = sb.tile([P, HCA], F16, name="xl_g", tag="xl_g",
                                       bufs=cfg.bufs_xl)
                        nc.gpsimd.indirect_dma_start(
                            out=xl_g[:], out_offset=None, in_=xl_full[:],
                            in_offset=bass.IndirectOffsetOnAxis(ap=src_i[:, t:t + 1], axis=0))
                        lr = sb.tile([P, HC], F16, name="lr", tag="lr", bufs=4)
                        lin = sb.tile([P, H], F32, name="lin", tag="lin", bufs=4)
                        for ci, (no, ns) in enumerate(ncs):
                            spc = ps2.tile([P, ns], F32, name="spc", tag="mm512",
                                           bufs=cfg.bufs_sp)
                            nc.tensor.matmul(out=spc[:, :ns],
                                             lhsT=de_b[:, t * P:(t + 1) * P],
                                             rhs=xr_blk[:, no:no + ns],
                                             start=True, stop=False)
                            nc.tensor.matmul(out=spc[:, :ns],
                                             lhsT=ident16[:],
                                             rhs=xl_g[:, no:no + ns],
                                             start=False, stop=True)
                            an = min(ns, HC - no)
                            if an > 0:
                                nc.scalar.activation(out=lr[:, no:no + an], in_=spc[:, :an],
                                                     func=mybir.ActivationFunctionType.Abs)
                            if no + ns > HC:
                                lo = max(HC - no, 0)
                                nc.vector.tensor_copy(out=lin[:, :H], in_=spc[:, lo:lo + H])
                        prod = sb.tile([P, HC], F16, name="prod", tag="prod", bufs=4)
                        nc.vector.tensor_tensor(out=prod[:], in0=lr[:],
                                                in1=att_sb[li][:],
                                                op=mybir.AluOpType.mult)
                        red = sb.tile([P, H], F32, name="red", tag="red", bufs=4)
                        nc.vector.tensor_reduce(
                            out=red[:], in_=prod[:].rearrange("p (h c) -> p h c", h=H),
                            axis=mybir.AxisListType.X, op=mybir.AluOpType.add)
                        logit = sb.tile([P, H], F32, name="logit", tag="logit", bufs=4)
                        nc.vector.tensor_tensor(out=logit[:], in0=red[:],
                                                in1=lin[:, :H],
                                                op=mybir.AluOpType.add)
                        w32 = sb.tile([P, H], F32, name="w32", tag="w32", bufs=4)
                        nc.scalar.activation(out=w32[:], in_=logit[:],
                                             func=mybir.ActivationFunctionType.Exp,
                                             bias=shift_col[:])
                        rsc = sb.tile([P, HC + H], F16, name="rsc", tag="rsc", bufs=6)
                        for h in range(H):
                            nc.vector.tensor_scalar(
                                out=rsc[:, h * C:(h + 1) * C], in0=xl_g[:, h * C:(h + 1) * C],
                                scalar1=w32[:, h:h + 1], scalar2=None,
                                op0=mybir.AluOpType.mult)
                        nc.vector.tensor_copy(out=rsc[:, HC:HC + H], in_=w32[:])
                        for (no, ns) in ucs:
                            nc.tensor.matmul(out=u_ps[:, no:no + ns],
                                             lhsT=oh_b[:, t * P:(t + 1) * P],
                                             rhs=rsc[:, no:no + ns],
                                             start=(t == 0), stop=(t == T - 1))

                    # ---------------- epilogue ----------------
                    dsb = sb.tile([P, H], F32, name="dsb", tag="dsb")
                    nc.vector.tensor_scalar(out=dsb[:], in0=u_ps[:, HC:HC + H],
                                            scalar1=1e-30, scalar2=None,
                                            op0=mybir.AluOpType.add)
                    recip = sb.tile([P, H], F32, name="recip", tag="recip")
                    nc.vector.reciprocal(out=recip[:], in_=dsb[:])
                    u16 = sb.tile([P, HC], F16, name="u16", tag="u16")
                    for h in range(H):
                        nc.scalar.activation(
                            out=u16[:, h * C:(h + 1) * C], in_=u_ps[:, h * C:(h + 1) * C],
                            func=mybir.ActivationFunctionType.Copy,
                            scale=recip[:, h:h + 1])
                    for kc, (fo, fs) in enumerate(_chunks(HC, P)):
                        tp2 = ps2.tile([P, P], F16, name="tp2", tag="small_ps", bufs=2)
                        nc.tensor.transpose(out=tp2[:fs, :], in_=u16[:, fo:fo + fs],
                                            identity=ident16[:])
                        hts = sb.tile([P, P], F16, name="hts", tag="hts")
                        nc.scalar.activation(out=hts[:fs, :bn], in_=tp2[:fs, :bn],
                                             func=mybir.ActivationFunctionType.Relu,
                                             bias=biasT_sb[li][:fs, kc:kc + 1],
                                             scale=1.0)
                        nc.sync.dma_start(
                            out=hT_dram[li][fo:fo + fs, b * P:b * P + bn],
                            in_=hts[:fs, :bn])

            # =========================================================
            for li, L in enumerate(cfg.layers):
                hT_dram.append(dram.tile([L.hc, SH], F16, name=f"hT{li}"))
            for li, L in enumerate(cfg.layers):
                ag_in, xr_sh = gemm_phase(li)
                xl_full = dram.tile([cfg.n_nodes, L.hca], F16, name=f"xl_full{li}",
                                    addr_space="Shared")
                nc.gpsimd.collective_compute(
                    "AllGather", mybir.AluOpType.bypass,
                    replica_groups=[list(range(cfg.n_cores))],
                    ins=[ag_in[:]], outs=[xl_full[:]])
                edge_phase(li, xl_full, xr_sh)

            # final linear
            kcs = _chunks(cfg.f_final, P)
            for m in range(NB):
                mo = m * P
                mn = min(P, SH - mo)
                pf = ps2.tile([P, 1], F32, name="pf", tag="small_ps", bufs=2)
                lhs = []
                for ki, (ko, ks) in enumerate(kcs):
                    lt = sb.tile([ks, P], F16, name="lhsTf", tag=f"lhsTf{ki}")
                    nc.sync.dma_start(out=lt[:, :mn], in_=hT_dram[-1][ko:ko + ks, mo:mo + mn])
                    lhs.append(lt)
                for ki, (ko, ks) in enumerate(kcs):
                    nc.tensor.matmul(out=pf[:mn, :], lhsT=lhs[ki][:, :mn],
                                     rhs=wf_sb[:ks, ki:ki + 1],
                                     start=(ki == 0), stop=(ki == len(kcs) - 1))
                of = sb.tile([P, 1], F32, name="of", tag="of")
                nc.scalar.activation(out=of[:mn, :], in_=pf[:mn, :],
                                     func=mybir.ActivationFunctionType.Identity,
                                     bias=bf_sb[:mn, :], scale=1.0)
                nc.sync.dma_start(out=out[mo:mo + mn, :], in_=of[:mn, :])

    nc.compile()
    return nc


# =====================================================================
# host-side data prep
# =====================================================================

def prep_host(inputs, cfg: GatCfg):
    N, SH, NB = cfg.n_nodes, cfg.shard, cfg.nblk
    x = np.asarray(inputs['x'], dtype=np.float32)
    ei = np.asarray(inputs['edge_index']).astype(np.int64)
    loop = np.arange(N, dtype=np.int64)
    src = np.concatenate([ei[0], loop])
    dst = np.concatenate([ei[1], loop])
    order = np.argsort(dst, kind='stable')
    src_s, dst_s = src[order], dst[order]

    cnt = np.zeros((cfg.n_cores, NB), dtype=np.int64)
    bounds = {}
    for c in range(cfg.n_cores):
        for b in range(NB):
            blk_lo = c * SH + b * P
            blk_hi = min(blk_lo + P, (c + 1) * SH)
            lo = np.searchsorted(dst_s, blk_lo)
            hi = np.searchsorted(dst_s, blk_hi)
            bounds[(c, b)] = (lo, hi, blk_lo)
            cnt[c, b] = hi - lo
    T = int((cnt.max() + P - 1) // P)
    cfg.T = T

    iarange = np.arange(P, dtype=np.int64)
    in_maps = []
    for c in range(cfg.n_cores):
        srcs = np.zeros((NB, P, T), dtype=np.int32)
        oh16 = np.zeros((NB, P, T * P), dtype=np.float16)
        de16 = np.zeros((NB, P, T * P), dtype=np.float16)
        for b in range(NB):
            lo, hi, blk_lo = bounds[(c, b)]
            ne = hi - lo
            s = np.zeros(T * P, dtype=np.int32)
            d = np.full(T * P, -1, dtype=np.int64)
            s[:ne] = src_s[lo:hi]
            d[:ne] = (dst_s[lo:hi] - blk_lo)
            srcs[b] = s.reshape(T, P).T
            oh_lin = (d[:, None] == iarange[None, :]).astype(np.float16)  # [T*P, P]
            oh_r = oh_lin.reshape(T, P, P)
            oh16[b] = oh_r.transpose(1, 0, 2).reshape(P, T * P)
            de16[b] = oh_r.transpose(2, 0, 1).reshape(P, T * P)
        xT = np.ascontiguousarray(x[c * SH:(c + 1) * SH, :].T).astype(np.float16)
        im = {'srcs': srcs, 'oh16': oh16, 'de16': de16, 'xT': xT}
        for li, L in enumerate(cfg.layers):
            Wl = np.asarray(inputs[f'Wl{li + 1}'], np.float32)
            Wr = np.asarray(inputs[f'Wr{li + 1}'], np.float32)
            att = np.asarray(inputs[f'att{li + 1}'], np.float32).reshape(1, -1)
            bias = np.asarray(inputs[f'b{li + 1}'], np.float32).reshape(-1)
            nkc = len(_chunks(L.hc, P))
            bT = np.zeros((P, nkc), dtype=np.float32)
            for kc, (fo, fs) in enumerate(_chunks(L.hc, P)):
                bT[:fs, kc] = bias[fo:fo + fs]
            att_bd = np.zeros((L.hc, L.heads), dtype=np.float32)
            for h in range(L.heads):
                att_bd[h * L.out_ch:(h + 1) * L.out_ch, h] = \
                    np.asarray(inputs[f'att{li + 1}'], np.float32)[h]
            wl_aug = np.concatenate([Wl, 0.6 * (Wl @ att_bd)], axis=1)
            wr_aug = np.concatenate([Wr, 0.6 * (Wr @ att_bd)], axis=1)
            im[f'wl{li}'] = wl_aug.astype(np.float16)
            im[f'wr{li}'] = wr_aug.astype(np.float16)
            im[f'att{li}'] = np.repeat(att * 0.4, P, axis=0).astype(np.float16)
            im[f'biasT{li}'] = bT
        wf_flat = np.asarray(inputs['Wf'], np.float32).reshape(-1)
        nkf = len(_chunks(cfg.f_final, P))
        wfp = np.zeros((P, nkf), dtype=np.float32)
        for ki, (ko, ks) in enumerate(_chunks(cfg.f_final, P)):
            wfp[:ks, ki] = wf_flat[ko:ko + ks]
        im['wf'] = wfp.astype(np.float16)
        im['bf_col'] = np.full((P, 1), np.asarray(inputs['bf'], np.float32).reshape(-1)[0],
                               dtype=np.float32)
        in_maps.append(im)
    return in_maps, T


_CACHE = {}


def kernel(**inputs) -> np.ndarray:
    cfg = GatCfg()
    in_maps, T = prep_host(inputs, cfg)
    key = ('v2', T)
    if key not in _CACHE:
        _CACHE[key] = build_gat(cfg)
    nc = _CACHE[key]
    res = bass_utils.run_bass_kernel_spmd(nc, in_maps, core_ids=list(range(cfg.n_cores)))
    out = np.concatenate([res.results[c]['out'] for c in range(cfg.n_cores)], axis=0)
    return out.astype(np.float32)
